# revision 17
# baseline (speedup 1.0000x reference)
"""Multi-head attention (B=4, N=2048, DIM=1024, H=16, HD=64) on 8 TRN2 cores.

Sharding: tensor-parallel over heads — 2 heads per core. The reference omits
the output projection, so each core's output is a disjoint 128-column slice of
the final [B, N, 1024]; no collectives are needed.

Per-core device kernel (bf16 compute, fp32 PSUM accumulation):
  - QKV projection from a single pass over x^T: q^T,k^T produced transposed
    [outch, tokens] (weights stationary), v produced natural [tokens, outch]
    (x tiles stationary).
  - scores^T = k^T.T @ q^T per (batch, head): K=64 contraction; head A lives
    on partitions 0-63 and head B on 64-127, so the two heads' matmuls
    row-tile the PE array and run concurrently.
  - exp split across two engines: most kt-chunks on ScalarE (table exp),
    the rest on DVE via a Schraudolph-style bit-trick (i16 = a*s + b,
    bitcast to bf16), which lands within the error tolerance and removes
    the ScalarE bottleneck.
  - out^T = [1 | v]^T @ expT accumulated over k tiles; row 0 is the softmax
    denominator. Tail: DVE fast reciprocal straight from PSUM row 0, GpSimd
    partition-broadcast, DVE multiply, GpSimd per-partition bias add, DMA out.
  - Projection work is emitted as micro-pieces interleaved into the score
    stream so the PE fills the exp-paced gaps instead of idling.
"""

import numpy as np
import ml_dtypes

import concourse.bacc as bacc
import concourse.mybir as mybir
from concourse.bass_utils import run_bass_kernel_spmd
from concourse.tile import TileContext

B, N, DIM, H = 4, 2048, 1024, 16
HD = DIM // H
SCALE = 1.0 / np.sqrt(HD)
TOK = B * N               # 8192 tokens
NCORES = 8
HPC = H // NCORES         # heads per core = 2

BF16 = mybir.dt.bfloat16
F32 = mybir.dt.float32
I16 = mybir.dt.int16
AF = mybir.ActivationFunctionType
ALU = mybir.AluOpType


NT = TOK // 512           # 16 token tiles of 512 for the projection
KT = 8                    # 1024 / 128 contraction tiles
QT = N // 512             # 4 q tiles per (b, h)
KTOK = N // 128           # 16 k-token tiles per (b, h)
VROW = 2 * (HD + 1)       # 130: [1 | vA | 1 | vB] per token tile

# Schraudolph bf16 exp: i16 = trunc(A*s + B), bitcast to bf16.
# A = 128/ln2 (SCALE folded into wq on host). B tuned for truncation.
SCH_A = 184.6650309
SCH_B = 16248.5

# exp split: odd kt chunks on DVE (bit-trick), even on ScalarE table exp —
# alternating parity so the two engines run concurrently within a chunk-pair
# AV matmuls trail the scores stream by AV_LAG chunks (even: chunk-pair
# granularity) so the PE switches stationary-weight streams half as often
AV_LAG = 4


def build_graph():
    nc = bacc.Bacc("TRN2", target_bir_lowering=False, debug=False)
    # host pre-arranges x/weights partition-major so every per-partition
    # tile block is one contiguous DMA descriptor (8KB for x tiles) instead
    # of 8x 1KB strided rows — the DMA engines are descriptor-rate bound
    xt = nc.declare_dram_parameter("xt", [128, NT * KT * 512], BF16,
                                   isOutput=False)
    wq = nc.declare_dram_parameter("wq", [128, KT * 128], BF16, isOutput=False)
    wk = nc.declare_dram_parameter("wk", [128, KT * 128], BF16, isOutput=False)
    wv = nc.declare_dram_parameter("wv", [128, KT * 128], BF16, isOutput=False)
    bqk = nc.declare_dram_parameter("bqk", [2 * HPC * HD, 1], F32, isOutput=False)
    bvq = nc.declare_dram_parameter("bvq", [HD + 1, HPC], F32, isOutput=False)
    out = nc.declare_dram_parameter("out", [HPC, B, N // 512, HD, 512], F32,
                                    isOutput=True)
    NTB = N // 512            # 4 proj token-tiles per batch
    KTOK_B = N // 128         # 16 k-token tiles per batch

    with TileContext(nc) as tc:
        with (
            tc.tile_pool(name="const", bufs=1) as constp,
            tc.tile_pool(name="qk", bufs=1) as qkp,
            tc.tile_pool(name="xin", bufs=4) as xinp,
            tc.tile_pool(name="exps", bufs=32) as expp,
            tc.tile_pool(name="outs", bufs=6) as outp,
            tc.tile_pool(name="rcs", bufs=2) as rcp,
        ):
            # ---- first x tile first: it gates the first matmul, and the
            # serial Sync trigger queue issues DMAs in program order
            xnt_tiles = {}

            def load(nt):
                xnt = xinp.tile([128, KT * 512], BF16, name="xnt")
                nc.sync.dma_start(out=xnt[:, :],
                                  in_=xt[:, nt * 4096:(nt + 1) * 4096])
                xnt_tiles[nt] = xnt

            # first x tile in two halves on the Sync queue: the first qk
            # group only reads kt 0-3, so projection starts after half the
            # transfer; everything else rides other engines' idle queues
            xnt0 = xinp.tile([128, KT * 512], BF16, name="xnt")
            for kh in range(2):
                nc.sync.dma_start(out=xnt0[:, kh * 2048:(kh + 1) * 2048],
                                  in_=xt[:, kh * 2048:(kh + 1) * 2048])
            xnt_tiles[0] = xnt0
            wq_s = constp.tile([128, KT * 128], BF16)
            wk_s = constp.tile([128, KT * 128], BF16)
            wv_s = constp.tile([128, KT * 128], BF16)
            nc.scalar.dma_start(out=wq_s[:, :], in_=wq[:, :])
            nc.scalar.dma_start(out=wk_s[:, :], in_=wk[:, :])
            nc.scalar.dma_start(out=wv_s[:, :], in_=wv[:, :])
            # bias transfers are many tiny descriptors: keep them off the
            # Sync queue so they don't delay the x tiles
            bqk_s = constp.tile([128, 2], F32)
            nc.gpsimd.dma_start(out=bqk_s[:, 0:1], in_=bqk[0:128, :])
            nc.gpsimd.dma_start(out=bqk_s[:, 1:2], in_=bqk[128:256, :])
            bvq_s = constp.tile([HD + 1, HPC], F32)
            nc.gpsimd.dma_start(out=bvq_s[:, :], in_=bvq[:, :])
            for nt0 in range(1, NTB):
                load(nt0)

            # per-batch activation tensors (lets attention on batch b start
            # as soon as batch b's projection tiles land)
            q_sb = [qkp.tile([128, N], BF16, name=f"q_sb{_b}") for _b in range(B)]
            k_sb = [qkp.tile([128, N], BF16, name=f"k_sb{_b}") for _b in range(B)]
            v_sb = [qkp.tile([128, KTOK_B * VROW], BF16, name=f"v_sb{_b}") for _b in range(B)]
            # warm tile on DVE (idle at startup) so warm-up matmuls begin
            # immediately; big v_sb memsets stay on GpSimd
            warm = constp.tile([128, 128], BF16)
            nc.vector.memset(warm[:, :], 0.25)
            for _b in range(B):
                nc.gpsimd.memset(v_sb[_b][:, :], 1.0)

            with (
                tc.tile_pool(name="qkps", bufs=1, space="PSUM") as qkps,
                tc.tile_pool(name="vps", bufs=1, space="PSUM") as vps,
                tc.tile_pool(name="sps", bufs=2, space="PSUM") as sps,
                tc.tile_pool(name="avps", bufs=1, space="PSUM") as avps,
            ):
                # PE p-state warm-up: ~6µs of junk matmuls keep the array
                # busy while the first x/w DMAs land, so real matmuls start
                # at full clock instead of the 0.65 GHz cold state
                wps = vps.tile([128, 128], F32, name="vp", tag="vp")
                NWARM = 48
                for wi in range(NWARM):
                    nc.tensor.matmul(wps[:, :], lhsT=warm[:, :], rhs=warm[:, :],
                                     start=(wi == 0), stop=(wi == NWARM - 1))

                qk_ps_live = {}

                def qk_mm(nt, mt, kh):
                    xnt = xnt_tiles[nt]
                    if kh == 0:
                        ps = qkps.tile([128, 512], F32, name="ps", tag="ps")
                        qk_ps_live[(nt, mt)] = ps
                    else:
                        ps = qk_ps_live[(nt, mt)]
                    wmt = wq_s if mt == 0 else wk_s
                    for kt in range(kh * 4, kh * 4 + 4):
                        nc.tensor.matmul(
                            ps[:, :],
                            lhsT=wmt[:, kt * 128:(kt + 1) * 128],
                            rhs=xnt[:, kt * 512:(kt + 1) * 512],
                            start=(kt == 0), stop=(kt == KT - 1))

                def qk_copy(nt, mt):
                    bb2, ntb2 = nt // NTB, nt % NTB
                    ps = qk_ps_live.pop((nt, mt))
                    dst = q_sb[bb2] if mt == 0 else k_sb[bb2]
                    nc.vector.tensor_scalar_add(
                        dst[:, ntb2 * 512:(ntb2 + 1) * 512], ps[:, :],
                        bqk_s[:, mt:mt + 1])

                def v_mm(nt, sub):
                    bb2, ntb2 = nt // NTB, nt % NTB
                    xnt = xnt_tiles[nt]
                    ttb = ntb2 * 4 + sub
                    vp = vps.tile([128, 128], F32, name="vp", tag="vp")
                    for kt in range(KT):
                        nc.tensor.matmul(
                            vp[:, :],
                            lhsT=xnt[:, kt * 512 + sub * 128:
                                     kt * 512 + (sub + 1) * 128],
                            rhs=wv_s[:, kt * 128:(kt + 1) * 128],
                            start=(kt == 0), stop=(kt == KT - 1))
                    # both heads in one strided copy:
                    # [128, 2, 64] -> v_sb cols [blk+1:blk+65],[blk+66:blk+130]
                    nc.vector.tensor_copy(
                        v_sb[bb2][:, ttb * VROW:(ttb + 1) * VROW]
                        .rearrange("p (h c) -> p h c", h=2)[:, :, 1:HD + 1],
                        vp.rearrange("p (h c) -> p h c", h=2))

                # ---- projection micro-pieces -------------------------------
                # Each piece is a small closure; cost is approximate PE-µs.
                def fill_pieces(bb, with_loads=True):
                    pieces = []
                    for ntb in range(NTB):
                        nt = bb * NTB + ntb
                        if with_loads:
                            pieces.append((0.05, lambda nt=nt: load(nt),
                                           bb, ntb))

                        def piece(cost, fn, *a):
                            return (cost, lambda fn=fn, a=a: fn(*a), bb, ntb)

                        # the last batch's q blocks for ntb>=1 are first read
                        # by wave (B-1, ntb): deferred there to fill the
                        # otherwise-idle chain-bound waves
                        defer_q = (bb == B - 1 and ntb >= 1)
                        if not defer_q:
                            pieces += [
                                piece(0.9, qk_mm, nt, 0, 0),
                                piece(0.9, qk_mm, nt, 0, 1),
                                piece(0.0, qk_copy, nt, 0),
                            ]
                        # v pieces spaced between the two qk groups so the
                        # qkps buffer's DVE copy has drained before reuse
                        pieces += [
                            piece(0.5, v_mm, nt, 0),
                            piece(0.5, v_mm, nt, 1),
                            piece(0.9, qk_mm, nt, 1, 0),
                            piece(0.9, qk_mm, nt, 1, 1),
                            piece(0.0, qk_copy, nt, 1),
                            piece(0.5, v_mm, nt, 2),
                            piece(0.5, v_mm, nt, 3),
                        ]
                    return pieces

                def emit_tail(pb, pqt, av):
                    # batch per engine so DVE never head-of-line blocks on
                    # a gpsimd broadcast
                    rcs, bcss, ots = [], [], []
                    for h in range(2):
                        rc = rcp.tile([1, 512], F32, name=f"rc{h}", tag=f"rc{h}")
                        nc.vector.reciprocal_approx_fast(rc[0:1, :], av[h][0:1, :])
                        rcs.append(rc)
                    for h in range(2):
                        bcs = rcp.tile([65, 512], F32, name=f"bcs{h}", tag=f"bcs{h}")
                        nc.gpsimd.partition_broadcast(bcs[:, :], rcs[h][0:1, :])
                        bcss.append(bcs)
                    for h in range(2):
                        ot = outp.tile([65, 512], F32)
                        nc.vector.tensor_mul(ot[0:65, :], av[h][0:65, :],
                                             bcss[h][0:65, :])
                        ots.append(ot)
                    for h in range(2):
                        ot2 = outp.tile([65, 512], F32, name="ot2", tag="ot2")
                        nc.scalar.activation(ot2[0:65, :], ots[h][0:65, :],
                                             AF.Identity, bias=bvq_s[:, h:h + 1])
                        nc.sync.dma_start(
                            out=out[h, pb, pqt, :, :],
                            in_=ot2[1:65, :])

                from collections import deque
                filler = deque()
                credit = [0.0]

                def pop_fill(add):
                    credit[0] += add
                    while filler and credit[0] > 0:
                        cost, fn, _, _ = filler.popleft()
                        credit[0] -= cost
                        fn()
                    if not filler:
                        credit[0] = 0.0

                def ensure_proj(bb, upto_ntb):
                    # scores of wave (bb, 0) chunk kt read k_sb[bb] columns
                    # written by block kt//4 — emission order IS the data
                    # order for the tile scheduler, so force-drain those
                    # pieces before emitting the consumer
                    while filler and any(
                            p[2] == bb and p[3] <= upto_ntb for p in filler):
                        cost, fn, _, _ = filler.popleft()
                        credit[0] -= cost
                        fn()

                # batch 0: nt0's q/k groups hand-scheduled for the startup
                # critical path — k accumulates in the vps bank so the
                # k-group needs no wait on the q-copy's qkps buffer
                qps0 = qkps.tile([128, 512], F32, name="ps", tag="ps")
                kps0 = vps.tile([128, 512], F32, name="vp", tag="vp")
                xnt00 = xnt_tiles[0]
                for kh in range(2):
                    for mt in range(2):
                        ps0 = qps0 if mt == 0 else kps0
                        wmt0 = wq_s if mt == 0 else wk_s
                        for kt in range(kh * 4, kh * 4 + 4):
                            nc.tensor.matmul(
                                ps0[:, :],
                                lhsT=wmt0[:, kt * 128:(kt + 1) * 128],
                                rhs=xnt00[:, kt * 512:(kt + 1) * 512],
                                start=(kt == 0), stop=(kt == KT - 1))
                for mt in range(2):
                    nc.vector.tensor_scalar_add(
                        (q_sb[0] if mt == 0 else k_sb[0])[:, 0:512],
                        (qps0 if mt == 0 else kps0)[:, :],
                        bqk_s[:, mt:mt + 1])
                b0_pieces = fill_pieces(0, with_loads=False)
                for cost, fn, _, _ in b0_pieces[3:5] + b0_pieces[8:10]:
                    fn()                                 # nt0 v pieces
                filler.extend(b0_pieces[10:])

                for b in range(B):
                    for qt in range(QT):
                        if qt == 1 and b + 1 < B:
                            filler.extend(fill_pieces(b + 1))
                        qcol = qt * 512
                        if b == B - 1 and qt >= 1:
                            # deferred q block for this wave (see fill_pieces)
                            nt_d = b * NTB + qt
                            qk_mm(nt_d, 0, 0)
                            qk_mm(nt_d, 0, 1)
                            qk_copy(nt_d, 0)
                        pav = [avps.tile([65, 512], F32, name=f"av{_h}",
                                         tag=f"av{_h}", bufs=1)
                               for _h in range(2)]

                        def av_mm(kt):
                            for h in range(2):
                                nc.tensor.matmul(
                                    pav[h][:, :],
                                    lhsT=v_sb[b][:, kt * VROW + h * (HD + 1):
                                                 kt * VROW + (h + 1) * (HD + 1)],
                                    rhs=echunks[kt][:, h * 512:(h + 1) * 512],
                                    start=(kt == 0), stop=(kt == KTOK_B - 1),
                                    skip_group_check=True)

                        echunks = []
                        for kt2 in range(0, KTOK_B, 2):
                            # two chunks of scores back-to-back: one stationary
                            # switch-in per pair instead of per chunk
                            for kt in (kt2, kt2 + 1):
                                kcol = kt * 128
                                if qt == 0:
                                    ensure_proj(b, kt // 4)
                                s2 = sps.tile([128, 1024], F32, name="s2",
                                              tag="s2")
                                for h in range(2):
                                    nc.tensor.matmul(
                                        s2[:, h * 512:(h + 1) * 512],
                                        lhsT=k_sb[b][h * 64:(h + 1) * 64,
                                                     kcol:kcol + 128],
                                        rhs=q_sb[b][h * 64:(h + 1) * 64,
                                                    qcol:qcol + 512],
                                        start=True, stop=True,
                                        tile_position=(h * 64, 0))
                                e2 = expp.tile([128, 1024], BF16, name="e2",
                                               tag="e2")
                                if kt % 2 == 1:
                                    nc.vector.tensor_scalar(
                                        out=e2[:, :].bitcast(I16), in0=s2[:, :],
                                        scalar1=SCH_A, scalar2=SCH_B,
                                        op0=ALU.mult, op1=ALU.add)
                                else:
                                    nc.scalar.activation(e2[:, :], s2[:, :],
                                                         AF.Exp)
                                echunks.append(e2)
                            # four AV matmuls trail as one block
                            if kt2 >= AV_LAG:
                                av_mm(kt2 - AV_LAG)
                                av_mm(kt2 - AV_LAG + 1)
                            gap_budget = 3.0 if (b, qt) == (0, 0) else 0.7
                            pop_fill(gap_budget if kt2 < KTOK_B - 2 else 0.0)
                        budget = 0.6 if (b, qt) != (B - 1, QT - 1) else 1e9
                        pop_fill(budget)
                        for kt in range(KTOK_B - AV_LAG, KTOK_B):
                            av_mm(kt)
                        emit_tail(b, qt, pav)
    nc.compile()
    return nc


_GRAPH = None


def _get_graph():
    global _GRAPH
    if _GRAPH is None:
        _GRAPH = build_graph()
    return _GRAPH


def _part_major(w_t):
    # [DIM, F] (x-dim major) -> [128, KT*F]: per-partition kt-blocks
    # contiguous so each partition is one DMA descriptor
    f = w_t.shape[1]
    return np.ascontiguousarray(
        w_t.reshape(KT, 128, f).transpose(1, 0, 2).reshape(128, KT * f))


def _make_in_maps(x, w_qkv, b_qkv):
    bf = ml_dtypes.bfloat16
    xt = x.reshape(TOK, DIM).T          # [DIM, TOK]
    # [128, nt, kt, 512]: per (partition, tile) an 8KB contiguous block
    xtr = np.ascontiguousarray(
        xt.reshape(KT, 128, NT, 512).transpose(1, 2, 0, 3)
        .reshape(128, NT * KT * 512)).astype(bf)
    in_maps = []
    for c in range(NCORES):
        hA, hB = HPC * c, HPC * c + 1
        rq = [w_qkv[h * HD:(h + 1) * HD] * SCALE for h in (hA, hB)]
        rk = [w_qkv[DIM + h * HD: DIM + (h + 1) * HD] for h in (hA, hB)]
        rv = [w_qkv[2 * DIM + h * HD: 2 * DIM + (h + 1) * HD] for h in (hA, hB)]
        wq_c = _part_major(np.concatenate(rq, axis=0).T).astype(bf)
        wk_c = _part_major(np.concatenate(rk, axis=0).T).astype(bf)
        wv_c = _part_major(np.concatenate(rv, axis=0).T).astype(bf)
        bq = [b_qkv[h * HD:(h + 1) * HD] * SCALE for h in (hA, hB)]
        bk = [b_qkv[DIM + h * HD: DIM + (h + 1) * HD] for h in (hA, hB)]
        bvc = [b_qkv[2 * DIM + h * HD: 2 * DIM + (h + 1) * HD] for h in (hA, hB)]
        bqk_c = np.concatenate(bq + bk).astype(np.float32).reshape(-1, 1)
        bvq_c = np.zeros((HD + 1, HPC), dtype=np.float32)
        for hh in range(HPC):
            bvq_c[1:HD + 1, hh] = bvc[hh]
        in_maps.append({"xt": xtr, "wq": wq_c, "wk": wk_c, "wv": wv_c,
                        "bqk": np.ascontiguousarray(bqk_c),
                        "bvq": bvq_c})
    return in_maps


def _run(x, w_qkv, b_qkv, trace=False, tmpdir=None):
    nc = _get_graph()
    in_maps = _make_in_maps(np.asarray(x, dtype=np.float32),
                            np.asarray(w_qkv, dtype=np.float32),
                            np.asarray(b_qkv, dtype=np.float32))
    res = run_bass_kernel_spmd(nc, in_maps, core_ids=list(range(NCORES)),
                               trace=trace, tmpdir=tmpdir)
    full = np.empty((B, N, DIM), dtype=np.float32)
    for c in range(NCORES):
        oc = res.results[c]["out"]          # [HPC, B, QT, HD, 512]
        # out[b, qt*512+j, (HPC*c+hh)*HD + d] = oc[hh, b, qt, d, j]
        full[:, :, c * HPC * HD:(c + 1) * HPC * HD] = \
            oc.transpose(1, 2, 4, 0, 3).reshape(B, N, HPC * HD)
    return full, res


def kernel(x, w_qkv, b_qkv):
    full, _ = _run(x, w_qkv, b_qkv, trace=False)
    return full



# revision 21
# speedup vs baseline: 1.0016x; 1.0016x over previous
"""Multi-head attention (B=4, N=2048, DIM=1024, H=16, HD=64) on 8 TRN2 cores.

Sharding: tensor-parallel over heads — 2 heads per core. The reference omits
the output projection, so each core's output is a disjoint 128-column slice of
the final [B, N, 1024]; no collectives are needed.

Per-core device kernel (bf16 compute, fp32 PSUM accumulation):
  - QKV projection from a single pass over x^T: q^T,k^T produced transposed
    [outch, tokens] (weights stationary), v produced natural [tokens, outch]
    (x tiles stationary).
  - scores^T = k^T.T @ q^T per (batch, head): K=64 contraction; head A lives
    on partitions 0-63 and head B on 64-127, so the two heads' matmuls
    row-tile the PE array and run concurrently.
  - exp split across two engines: most kt-chunks on ScalarE (table exp),
    the rest on DVE via a Schraudolph-style bit-trick (i16 = a*s + b,
    bitcast to bf16), which lands within the error tolerance and removes
    the ScalarE bottleneck.
  - out^T = [1 | v]^T @ expT accumulated over k tiles; row 0 is the softmax
    denominator. Tail: DVE fast reciprocal straight from PSUM row 0, GpSimd
    partition-broadcast, DVE multiply, GpSimd per-partition bias add, DMA out.
  - Projection work is emitted as micro-pieces interleaved into the score
    stream so the PE fills the exp-paced gaps instead of idling.
"""

import numpy as np
import ml_dtypes

import concourse.bacc as bacc
import concourse.mybir as mybir
from concourse.bass_utils import run_bass_kernel_spmd
from concourse.tile import TileContext

B, N, DIM, H = 4, 2048, 1024, 16
HD = DIM // H
SCALE = 1.0 / np.sqrt(HD)
TOK = B * N               # 8192 tokens
NCORES = 8
HPC = H // NCORES         # heads per core = 2

BF16 = mybir.dt.bfloat16
F32 = mybir.dt.float32
I16 = mybir.dt.int16
AF = mybir.ActivationFunctionType
ALU = mybir.AluOpType


NT = TOK // 512           # 16 token tiles of 512 for the projection
KT = 8                    # 1024 / 128 contraction tiles
QT = N // 512             # 4 q tiles per (b, h)
KTOK = N // 128           # 16 k-token tiles per (b, h)
VROW = 2 * (HD + 1)       # 130: [1 | vA | 1 | vB] per token tile

# Schraudolph bf16 exp: i16 = trunc(A*s + B), bitcast to bf16.
# A = 128/ln2 (SCALE folded into wq on host). B tuned for truncation.
SCH_A = 184.6650309
SCH_B = 16248.5

# exp split: odd kt chunks on DVE (bit-trick), even on ScalarE table exp —
# alternating parity so the two engines run concurrently within a chunk-pair
# AV matmuls trail the scores stream by AV_LAG chunks (even: chunk-pair
# granularity) so the PE switches stationary-weight streams half as often
AV_LAG = 4


def build_graph():
    nc = bacc.Bacc("TRN2", target_bir_lowering=False, debug=False)
    # host pre-arranges x/weights partition-major so every per-partition
    # tile block is one contiguous DMA descriptor (8KB for x tiles) instead
    # of 8x 1KB strided rows — the DMA engines are descriptor-rate bound
    xt = nc.declare_dram_parameter("xt", [128, NT * KT * 512], BF16,
                                   isOutput=False)
    wq = nc.declare_dram_parameter("wq", [128, KT * 128], BF16, isOutput=False)
    wk = nc.declare_dram_parameter("wk", [128, KT * 128], BF16, isOutput=False)
    wv = nc.declare_dram_parameter("wv", [128, KT * 128], BF16, isOutput=False)
    bqk = nc.declare_dram_parameter("bqk", [2 * HPC * HD, 1], F32, isOutput=False)
    bvq = nc.declare_dram_parameter("bvq", [HD + 1, HPC], F32, isOutput=False)
    out = nc.declare_dram_parameter("out", [HPC, B, N // 512, HD, 512], F32,
                                    isOutput=True)
    NTB = N // 512            # 4 proj token-tiles per batch
    KTOK_B = N // 128         # 16 k-token tiles per batch

    with TileContext(nc) as tc:
        with (
            tc.tile_pool(name="const", bufs=1) as constp,
            tc.tile_pool(name="qk", bufs=1) as qkp,
            tc.tile_pool(name="xin", bufs=4) as xinp,
            tc.tile_pool(name="exps", bufs=32) as expp,
            tc.tile_pool(name="outs", bufs=6) as outp,
            tc.tile_pool(name="rcs", bufs=2) as rcp,
        ):
            # ---- first x tile first: it gates the first matmul, and the
            # serial Sync trigger queue issues DMAs in program order
            xnt_tiles = {}

            def load(nt):
                xnt = xinp.tile([128, KT * 512], BF16, name="xnt")
                nc.sync.dma_start(out=xnt[:, :],
                                  in_=xt[:, nt * 4096:(nt + 1) * 4096])
                xnt_tiles[nt] = xnt

            # startup-critical bytes in dependency order on the 16-engine
            # Sync queue (other engines' queues measured ~4x slower): the
            # first q-group needs xnt0 kt0-3 + wq; the k-group adds wk and
            # the second x half; wv/v_mm come later
            xnt0 = xinp.tile([128, KT * 512], BF16, name="xnt")
            wq_s = constp.tile([128, KT * 128], BF16)
            wk_s = constp.tile([128, KT * 128], BF16)
            wv_s = constp.tile([128, KT * 128], BF16)
            nc.sync.dma_start(out=xnt0[:, 0:2048], in_=xt[:, 0:2048])
            nc.sync.dma_start(out=wq_s[:, :], in_=wq[:, :])
            nc.sync.dma_start(out=wk_s[:, :], in_=wk[:, :])
            nc.sync.dma_start(out=xnt0[:, 2048:4096], in_=xt[:, 2048:4096])
            nc.sync.dma_start(out=wv_s[:, :], in_=wv[:, :])
            xnt_tiles[0] = xnt0
            # bias transfers are many tiny descriptors: keep them off the
            # Sync queue so they don't delay the x tiles
            bqk_s = constp.tile([128, 2], F32)
            nc.gpsimd.dma_start(out=bqk_s[:, 0:1], in_=bqk[0:128, :])
            nc.gpsimd.dma_start(out=bqk_s[:, 1:2], in_=bqk[128:256, :])
            bvq_s = constp.tile([HD + 1, HPC], F32)
            nc.gpsimd.dma_start(out=bvq_s[:, :], in_=bvq[:, :])
            for nt0 in range(1, NTB):
                load(nt0)

            # per-batch activation tensors (lets attention on batch b start
            # as soon as batch b's projection tiles land)
            q_sb = [qkp.tile([128, N], BF16, name=f"q_sb{_b}") for _b in range(B)]
            k_sb = [qkp.tile([128, N], BF16, name=f"k_sb{_b}") for _b in range(B)]
            v_sb = [qkp.tile([128, KTOK_B * VROW], BF16, name=f"v_sb{_b}") for _b in range(B)]
            # memsets on GpSimd (idle at startup) so DVE is free immediately;
            # warm tile first — it gates the PE clock warm-up
            warm = constp.tile([128, 128], BF16)
            nc.gpsimd.memset(warm[:, :], 0.25)
            for _b in range(B):
                nc.gpsimd.memset(v_sb[_b][:, :], 1.0)

            with (
                tc.tile_pool(name="qkps", bufs=1, space="PSUM") as qkps,
                tc.tile_pool(name="vps", bufs=1, space="PSUM") as vps,
                tc.tile_pool(name="sps", bufs=2, space="PSUM") as sps,
                tc.tile_pool(name="avps", bufs=1, space="PSUM") as avps,
            ):
                # PE p-state warm-up: ~6µs of junk matmuls keep the array
                # busy while the first x/w DMAs land, so real matmuls start
                # at full clock instead of the 0.65 GHz cold state
                wps = vps.tile([128, 128], F32, name="vp", tag="vp")
                NWARM = 80
                for wi in range(NWARM):
                    nc.tensor.matmul(wps[:, :], lhsT=warm[:, :], rhs=warm[:, :],
                                     start=(wi == 0), stop=(wi == NWARM - 1))

                qk_ps_live = {}

                def qk_mm(nt, mt, kh):
                    xnt = xnt_tiles[nt]
                    if kh == 0:
                        ps = qkps.tile([128, 512], F32, name="ps", tag="ps")
                        qk_ps_live[(nt, mt)] = ps
                    else:
                        ps = qk_ps_live[(nt, mt)]
                    wmt = wq_s if mt == 0 else wk_s
                    for kt in range(kh * 4, kh * 4 + 4):
                        nc.tensor.matmul(
                            ps[:, :],
                            lhsT=wmt[:, kt * 128:(kt + 1) * 128],
                            rhs=xnt[:, kt * 512:(kt + 1) * 512],
                            start=(kt == 0), stop=(kt == KT - 1))

                def qk_copy(nt, mt):
                    bb2, ntb2 = nt // NTB, nt % NTB
                    ps = qk_ps_live.pop((nt, mt))
                    dst = q_sb[bb2] if mt == 0 else k_sb[bb2]
                    nc.vector.tensor_scalar_add(
                        dst[:, ntb2 * 512:(ntb2 + 1) * 512], ps[:, :],
                        bqk_s[:, mt:mt + 1])

                def v_mm(nt, sub):
                    bb2, ntb2 = nt // NTB, nt % NTB
                    xnt = xnt_tiles[nt]
                    ttb = ntb2 * 4 + sub
                    vp = vps.tile([128, 128], F32, name="vp", tag="vp")
                    for kt in range(KT):
                        nc.tensor.matmul(
                            vp[:, :],
                            lhsT=xnt[:, kt * 512 + sub * 128:
                                     kt * 512 + (sub + 1) * 128],
                            rhs=wv_s[:, kt * 128:(kt + 1) * 128],
                            start=(kt == 0), stop=(kt == KT - 1))
                    # both heads in one strided copy:
                    # [128, 2, 64] -> v_sb cols [blk+1:blk+65],[blk+66:blk+130]
                    nc.vector.tensor_copy(
                        v_sb[bb2][:, ttb * VROW:(ttb + 1) * VROW]
                        .rearrange("p (h c) -> p h c", h=2)[:, :, 1:HD + 1],
                        vp.rearrange("p (h c) -> p h c", h=2))

                # ---- projection micro-pieces -------------------------------
                # Each piece is a small closure; cost is approximate PE-µs.
                def fill_pieces(bb, with_loads=True):
                    pieces = []
                    for ntb in range(NTB):
                        nt = bb * NTB + ntb
                        if with_loads:
                            pieces.append((0.05, lambda nt=nt: load(nt),
                                           bb, ntb))

                        def piece(cost, fn, *a):
                            return (cost, lambda fn=fn, a=a: fn(*a), bb, ntb)

                        # the last batch's q blocks for ntb>=1 are first read
                        # by wave (B-1, ntb): deferred there to fill the
                        # otherwise-idle chain-bound waves
                        defer_q = (bb == B - 1 and ntb >= 1)
                        if not defer_q:
                            pieces += [
                                piece(0.9, qk_mm, nt, 0, 0),
                                piece(0.9, qk_mm, nt, 0, 1),
                                piece(0.0, qk_copy, nt, 0),
                            ]
                        # v pieces spaced between the two qk groups so the
                        # qkps buffer's DVE copy has drained before reuse
                        pieces += [
                            piece(0.5, v_mm, nt, 0),
                            piece(0.5, v_mm, nt, 1),
                            piece(0.9, qk_mm, nt, 1, 0),
                            piece(0.9, qk_mm, nt, 1, 1),
                            piece(0.0, qk_copy, nt, 1),
                            piece(0.5, v_mm, nt, 2),
                            piece(0.5, v_mm, nt, 3),
                        ]
                    return pieces

                def emit_tail(pb, pqt, av):
                    # batch per engine so DVE never head-of-line blocks on
                    # a gpsimd broadcast
                    rcs, bcss, ots = [], [], []
                    for h in range(2):
                        rc = rcp.tile([1, 512], F32, name=f"rc{h}", tag=f"rc{h}")
                        nc.vector.reciprocal_approx_fast(rc[0:1, :], av[h][0:1, :])
                        rcs.append(rc)
                    for h in range(2):
                        bcs = rcp.tile([65, 512], F32, name=f"bcs{h}", tag=f"bcs{h}")
                        nc.gpsimd.partition_broadcast(bcs[:, :], rcs[h][0:1, :])
                        bcss.append(bcs)
                    for h in range(2):
                        ot = outp.tile([65, 512], F32)
                        nc.vector.tensor_mul(ot[0:65, :], av[h][0:65, :],
                                             bcss[h][0:65, :])
                        ots.append(ot)
                    for h in range(2):
                        ot2 = outp.tile([65, 512], F32, name="ot2", tag="ot2")
                        nc.scalar.activation(ot2[0:65, :], ots[h][0:65, :],
                                             AF.Identity, bias=bvq_s[:, h:h + 1])
                        nc.sync.dma_start(
                            out=out[h, pb, pqt, :, :],
                            in_=ot2[1:65, :])

                from collections import deque
                filler = deque()
                credit = [0.0]

                def pop_fill(add):
                    credit[0] += add
                    while filler and credit[0] > 0:
                        cost, fn, _, _ = filler.popleft()
                        credit[0] -= cost
                        fn()
                    if not filler:
                        credit[0] = 0.0

                def ensure_proj(bb, upto_ntb):
                    # scores of wave (bb, 0) chunk kt read k_sb[bb] columns
                    # written by block kt//4 — emission order IS the data
                    # order for the tile scheduler, so force-drain those
                    # pieces before emitting the consumer
                    while filler and any(
                            p[2] == bb and p[3] <= upto_ntb for p in filler):
                        cost, fn, _, _ = filler.popleft()
                        credit[0] -= cost
                        fn()

                # batch 0: nt0's q/k groups hand-scheduled for the startup
                # critical path — k accumulates in the vps bank so the
                # k-group needs no wait on the q-copy's qkps buffer
                qps0 = qkps.tile([128, 512], F32, name="ps", tag="ps")
                kps0 = vps.tile([128, 512], F32, name="vp", tag="vp")
                xnt00 = xnt_tiles[0]
                for kh in range(2):
                    for mt in range(2):
                        ps0 = qps0 if mt == 0 else kps0
                        wmt0 = wq_s if mt == 0 else wk_s
                        for kt in range(kh * 4, kh * 4 + 4):
                            nc.tensor.matmul(
                                ps0[:, :],
                                lhsT=wmt0[:, kt * 128:(kt + 1) * 128],
                                rhs=xnt00[:, kt * 512:(kt + 1) * 512],
                                start=(kt == 0), stop=(kt == KT - 1))
                for mt in range(2):
                    nc.vector.tensor_scalar_add(
                        (q_sb[0] if mt == 0 else k_sb[0])[:, 0:512],
                        (qps0 if mt == 0 else kps0)[:, :],
                        bqk_s[:, mt:mt + 1])
                b0_pieces = fill_pieces(0, with_loads=False)
                for cost, fn, _, _ in b0_pieces[3:5] + b0_pieces[8:10]:
                    fn()                                 # nt0 v pieces
                filler.extend(b0_pieces[10:])

                for b in range(B):
                    for qt in range(QT):
                        if qt == 1 and b + 1 < B:
                            filler.extend(fill_pieces(b + 1))
                        qcol = qt * 512
                        if b == B - 1 and qt >= 1:
                            # deferred q block for this wave (see fill_pieces)
                            nt_d = b * NTB + qt
                            qk_mm(nt_d, 0, 0)
                            qk_mm(nt_d, 0, 1)
                            qk_copy(nt_d, 0)
                        pav = [avps.tile([65, 512], F32, name=f"av{_h}",
                                         tag=f"av{_h}", bufs=1)
                               for _h in range(2)]

                        def av_mm(kt):
                            for h in range(2):
                                nc.tensor.matmul(
                                    pav[h][:, :],
                                    lhsT=v_sb[b][:, kt * VROW + h * (HD + 1):
                                                 kt * VROW + (h + 1) * (HD + 1)],
                                    rhs=echunks[kt][:, h * 512:(h + 1) * 512],
                                    start=(kt == 0), stop=(kt == KTOK_B - 1),
                                    skip_group_check=True)

                        echunks = []
                        for kt2 in range(0, KTOK_B, 2):
                            # two chunks of scores back-to-back: one stationary
                            # switch-in per pair instead of per chunk
                            for kt in (kt2, kt2 + 1):
                                kcol = kt * 128
                                if qt == 0:
                                    ensure_proj(b, kt // 4)
                                s2 = sps.tile([128, 1024], F32, name="s2",
                                              tag="s2")
                                for h in range(2):
                                    nc.tensor.matmul(
                                        s2[:, h * 512:(h + 1) * 512],
                                        lhsT=k_sb[b][h * 64:(h + 1) * 64,
                                                     kcol:kcol + 128],
                                        rhs=q_sb[b][h * 64:(h + 1) * 64,
                                                    qcol:qcol + 512],
                                        start=True, stop=True,
                                        tile_position=(h * 64, 0))
                                e2 = expp.tile([128, 1024], BF16, name="e2",
                                               tag="e2")
                                # last wave: no filler left, its exp chain is
                                # the endgame — rebalance toward DVE so both
                                # exp engines finish together (~6.2us each)
                                last_wave = (b, qt) == (B - 1, QT - 1)
                                use_dve = (kt % 3 != 0) if last_wave \
                                    else (kt % 2 == 1)
                                if use_dve:
                                    nc.vector.tensor_scalar(
                                        out=e2[:, :].bitcast(I16), in0=s2[:, :],
                                        scalar1=SCH_A, scalar2=SCH_B,
                                        op0=ALU.mult, op1=ALU.add)
                                else:
                                    nc.scalar.activation(e2[:, :], s2[:, :],
                                                         AF.Exp)
                                echunks.append(e2)
                            # four AV matmuls trail as one block
                            if kt2 >= AV_LAG:
                                av_mm(kt2 - AV_LAG)
                                av_mm(kt2 - AV_LAG + 1)
                            gap_budget = 3.0 if (b, qt) == (0, 0) else 0.7
                            pop_fill(gap_budget if kt2 < KTOK_B - 2 else 0.0)
                        budget = 0.6 if (b, qt) != (B - 1, QT - 1) else 1e9
                        pop_fill(budget)
                        for kt in range(KTOK_B - AV_LAG, KTOK_B):
                            av_mm(kt)
                        emit_tail(b, qt, pav)
    nc.compile()
    return nc


_GRAPH = None


def _get_graph():
    global _GRAPH
    if _GRAPH is None:
        _GRAPH = build_graph()
    return _GRAPH


def _part_major(w_t):
    # [DIM, F] (x-dim major) -> [128, KT*F]: per-partition kt-blocks
    # contiguous so each partition is one DMA descriptor
    f = w_t.shape[1]
    return np.ascontiguousarray(
        w_t.reshape(KT, 128, f).transpose(1, 0, 2).reshape(128, KT * f))


def _make_in_maps(x, w_qkv, b_qkv):
    bf = ml_dtypes.bfloat16
    xt = x.reshape(TOK, DIM).T          # [DIM, TOK]
    # [128, nt, kt, 512]: per (partition, tile) an 8KB contiguous block
    xtr = np.ascontiguousarray(
        xt.reshape(KT, 128, NT, 512).transpose(1, 2, 0, 3)
        .reshape(128, NT * KT * 512)).astype(bf)
    in_maps = []
    for c in range(NCORES):
        hA, hB = HPC * c, HPC * c + 1
        rq = [w_qkv[h * HD:(h + 1) * HD] * SCALE for h in (hA, hB)]
        rk = [w_qkv[DIM + h * HD: DIM + (h + 1) * HD] for h in (hA, hB)]
        rv = [w_qkv[2 * DIM + h * HD: 2 * DIM + (h + 1) * HD] for h in (hA, hB)]
        wq_c = _part_major(np.concatenate(rq, axis=0).T).astype(bf)
        wk_c = _part_major(np.concatenate(rk, axis=0).T).astype(bf)
        wv_c = _part_major(np.concatenate(rv, axis=0).T).astype(bf)
        bq = [b_qkv[h * HD:(h + 1) * HD] * SCALE for h in (hA, hB)]
        bk = [b_qkv[DIM + h * HD: DIM + (h + 1) * HD] for h in (hA, hB)]
        bvc = [b_qkv[2 * DIM + h * HD: 2 * DIM + (h + 1) * HD] for h in (hA, hB)]
        bqk_c = np.concatenate(bq + bk).astype(np.float32).reshape(-1, 1)
        bvq_c = np.zeros((HD + 1, HPC), dtype=np.float32)
        for hh in range(HPC):
            bvq_c[1:HD + 1, hh] = bvc[hh]
        in_maps.append({"xt": xtr, "wq": wq_c, "wk": wk_c, "wv": wv_c,
                        "bqk": np.ascontiguousarray(bqk_c),
                        "bvq": bvq_c})
    return in_maps


def _run(x, w_qkv, b_qkv, trace=False, tmpdir=None):
    nc = _get_graph()
    in_maps = _make_in_maps(np.asarray(x, dtype=np.float32),
                            np.asarray(w_qkv, dtype=np.float32),
                            np.asarray(b_qkv, dtype=np.float32))
    res = run_bass_kernel_spmd(nc, in_maps, core_ids=list(range(NCORES)),
                               trace=trace, tmpdir=tmpdir)
    full = np.empty((B, N, DIM), dtype=np.float32)
    for c in range(NCORES):
        oc = res.results[c]["out"]          # [HPC, B, QT, HD, 512]
        # out[b, qt*512+j, (HPC*c+hh)*HD + d] = oc[hh, b, qt, d, j]
        full[:, :, c * HPC * HD:(c + 1) * HPC * HD] = \
            oc.transpose(1, 2, 4, 0, 3).reshape(B, N, HPC * HD)
    return full, res


def kernel(x, w_qkv, b_qkv):
    full, _ = _run(x, w_qkv, b_qkv, trace=False)
    return full



# revision 25
# speedup vs baseline: 1.0161x; 1.0145x over previous
"""Multi-head attention (B=4, N=2048, DIM=1024, H=16, HD=64) on 8 TRN2 cores.

Sharding: tensor-parallel over heads — 2 heads per core. The reference omits
the output projection, so each core's output is a disjoint 128-column slice of
the final [B, N, 1024]; no collectives are needed.

Per-core device kernel (bf16 compute, fp32 PSUM accumulation):
  - QKV projection from a single pass over x^T: q^T,k^T produced transposed
    [outch, tokens] (weights stationary), v produced natural [tokens, outch]
    (x tiles stationary).
  - scores^T = k^T.T @ q^T per (batch, head): K=64 contraction; head A lives
    on partitions 0-63 and head B on 64-127, so the two heads' matmuls
    row-tile the PE array and run concurrently.
  - exp split across two engines: most kt-chunks on ScalarE (table exp),
    the rest on DVE via a Schraudolph-style bit-trick (i16 = a*s + b,
    bitcast to bf16), which lands within the error tolerance and removes
    the ScalarE bottleneck.
  - out^T = [1 | v]^T @ expT accumulated over k tiles; row 0 is the softmax
    denominator. Tail: DVE fast reciprocal straight from PSUM row 0, GpSimd
    partition-broadcast, DVE multiply, GpSimd per-partition bias add, DMA out.
  - Projection work is emitted as micro-pieces interleaved into the score
    stream so the PE fills the exp-paced gaps instead of idling.
"""

import numpy as np
import ml_dtypes

import concourse.bacc as bacc
import concourse.mybir as mybir
from concourse.bass_utils import run_bass_kernel_spmd
from concourse.tile import TileContext

B, N, DIM, H = 4, 2048, 1024, 16
HD = DIM // H
SCALE = 1.0 / np.sqrt(HD)
TOK = B * N               # 8192 tokens
NCORES = 8
HPC = H // NCORES         # heads per core = 2

BF16 = mybir.dt.bfloat16
F32 = mybir.dt.float32
I16 = mybir.dt.int16
AF = mybir.ActivationFunctionType
ALU = mybir.AluOpType


NT = TOK // 512           # 16 token tiles of 512 for the projection
KT = 8                    # 1024 / 128 contraction tiles
QT = N // 512             # 4 q tiles per (b, h)
KTOK = N // 128           # 16 k-token tiles per (b, h)
VROW = 2 * (HD + 1)       # 130: [1 | vA | 1 | vB] per token tile

# Schraudolph bf16 exp: i16 = trunc(A*s + B), bitcast to bf16.
# A = 128/ln2 (SCALE folded into wq on host). B tuned for truncation.
SCH_A = 184.6650309
SCH_B = 16248.5

# exp split: odd kt chunks on DVE (bit-trick), even on ScalarE table exp —
# alternating parity so the two engines run concurrently within a chunk-pair
# AV matmuls trail the scores stream by AV_LAG chunks (even: chunk-pair
# granularity) so the PE switches stationary-weight streams half as often
AV_LAG = 6


def build_graph():
    nc = bacc.Bacc("TRN2", target_bir_lowering=False, debug=False)
    # host pre-arranges x/weights partition-major so every per-partition
    # tile block is one contiguous DMA descriptor (8KB for x tiles) instead
    # of 8x 1KB strided rows — the DMA engines are descriptor-rate bound
    xt = nc.declare_dram_parameter("xt", [128, NT * KT * 512], BF16,
                                   isOutput=False)
    wq = nc.declare_dram_parameter("wq", [128, KT * 128], BF16, isOutput=False)
    wk = nc.declare_dram_parameter("wk", [128, KT * 128], BF16, isOutput=False)
    wv = nc.declare_dram_parameter("wv", [128, KT * 128], BF16, isOutput=False)
    bqk = nc.declare_dram_parameter("bqk", [2 * HPC * HD, 1], F32, isOutput=False)
    bvq = nc.declare_dram_parameter("bvq", [HD + 1, HPC], F32, isOutput=False)
    out = nc.declare_dram_parameter("out", [HPC, B, N // 512, HD, 512], F32,
                                    isOutput=True)
    NTB = N // 512            # 4 proj token-tiles per batch
    KTOK_B = N // 128         # 16 k-token tiles per batch

    with TileContext(nc) as tc:
        with (
            tc.tile_pool(name="const", bufs=1) as constp,
            tc.tile_pool(name="qk", bufs=1) as qkp,
            tc.tile_pool(name="xin", bufs=4) as xinp,
            tc.tile_pool(name="exps", bufs=32) as expp,
            tc.tile_pool(name="outs", bufs=6) as outp,
            tc.tile_pool(name="rcs", bufs=2) as rcp,
        ):
            # ---- first x tile first: it gates the first matmul, and the
            # serial Sync trigger queue issues DMAs in program order
            xnt_tiles = {}

            def load(nt):
                xnt = xinp.tile([128, KT * 512], BF16, name="xnt")
                nc.sync.dma_start(out=xnt[:, :],
                                  in_=xt[:, nt * 4096:(nt + 1) * 4096])
                xnt_tiles[nt] = xnt

            # startup-critical bytes in dependency order on the 16-engine
            # Sync queue (other engines' queues measured ~4x slower): the
            # first q-group needs xnt0 kt0-3 + wq; the k-group adds wk and
            # the second x half; wv/v_mm come later
            xnt0 = xinp.tile([128, KT * 512], BF16, name="xnt")
            wq_s = constp.tile([128, KT * 128], BF16)
            wk_s = constp.tile([128, KT * 128], BF16)
            wv_s = constp.tile([128, KT * 128], BF16)
            # warm tile first on the GpSimd queue: it gates the PE clock
            # warm-up and must land before the bias DMA triggers
            warm = constp.tile([128, 128], BF16)
            nc.gpsimd.memset(warm[:, :], 0.25)
            nc.sync.dma_start(out=xnt0[:, 0:2048], in_=xt[:, 0:2048])
            nc.sync.dma_start(out=wq_s[:, :], in_=wq[:, :])
            nc.sync.dma_start(out=wk_s[:, :], in_=wk[:, :])
            nc.sync.dma_start(out=xnt0[:, 2048:4096], in_=xt[:, 2048:4096])
            nc.sync.dma_start(out=wv_s[:, :], in_=wv[:, :])
            xnt_tiles[0] = xnt0
            # bias transfers are many tiny descriptors: keep them off the
            # Sync queue so they don't delay the x tiles
            bqk_s = constp.tile([128, 2], F32)
            nc.gpsimd.dma_start(out=bqk_s[:, 0:1], in_=bqk[0:128, :])
            nc.gpsimd.dma_start(out=bqk_s[:, 1:2], in_=bqk[128:256, :])
            bvq_s = constp.tile([HD + 1, HPC], F32)
            nc.gpsimd.dma_start(out=bvq_s[:, :], in_=bvq[:, :])
            for nt0 in range(1, NTB):
                load(nt0)

            # per-batch activation tensors (lets attention on batch b start
            # as soon as batch b's projection tiles land)
            q_sb = [qkp.tile([128, N], BF16, name=f"q_sb{_b}") for _b in range(B)]
            k_sb = [qkp.tile([128, N], BF16, name=f"k_sb{_b}") for _b in range(B)]
            v_sb = [qkp.tile([128, KTOK_B * VROW], BF16, name=f"v_sb{_b}") for _b in range(B)]
            # memsets on GpSimd (idle at startup) so DVE is free immediately
            for _b in range(B):
                nc.gpsimd.memset(v_sb[_b][:, :], 1.0)

            with (
                tc.tile_pool(name="qkps", bufs=1, space="PSUM") as qkps,
                tc.tile_pool(name="vps", bufs=1, space="PSUM") as vps,
                tc.tile_pool(name="sps", bufs=2, space="PSUM") as sps,
                tc.tile_pool(name="avps", bufs=1, space="PSUM") as avps,
            ):
                # PE p-state warm-up: ~6µs of junk matmuls keep the array
                # busy while the first x/w DMAs land, so real matmuls start
                # at full clock instead of the 0.65 GHz cold state
                wps = vps.tile([128, 128], F32, name="vp", tag="vp")
                NWARM = 52
                for wi in range(NWARM):
                    nc.tensor.matmul(wps[:, :], lhsT=warm[:, :], rhs=warm[:, :],
                                     start=(wi == 0), stop=(wi == NWARM - 1))

                qk_ps_live = {}

                def qk_mm(nt, mt, kh):
                    xnt = xnt_tiles[nt]
                    if kh == 0:
                        ps = qkps.tile([128, 512], F32, name="ps", tag="ps")
                        qk_ps_live[(nt, mt)] = ps
                    else:
                        ps = qk_ps_live[(nt, mt)]
                    wmt = wq_s if mt == 0 else wk_s
                    for kt in range(kh * 4, kh * 4 + 4):
                        nc.tensor.matmul(
                            ps[:, :],
                            lhsT=wmt[:, kt * 128:(kt + 1) * 128],
                            rhs=xnt[:, kt * 512:(kt + 1) * 512],
                            start=(kt == 0), stop=(kt == KT - 1))

                def qk_copy(nt, mt):
                    bb2, ntb2 = nt // NTB, nt % NTB
                    ps = qk_ps_live.pop((nt, mt))
                    dst = q_sb[bb2] if mt == 0 else k_sb[bb2]
                    nc.vector.tensor_scalar_add(
                        dst[:, ntb2 * 512:(ntb2 + 1) * 512], ps[:, :],
                        bqk_s[:, mt:mt + 1])

                def v_mm(nt, sub):
                    bb2, ntb2 = nt // NTB, nt % NTB
                    xnt = xnt_tiles[nt]
                    ttb = ntb2 * 4 + sub
                    vp = vps.tile([128, 128], F32, name="vp", tag="vp")
                    for kt in range(KT):
                        nc.tensor.matmul(
                            vp[:, :],
                            lhsT=xnt[:, kt * 512 + sub * 128:
                                     kt * 512 + (sub + 1) * 128],
                            rhs=wv_s[:, kt * 128:(kt + 1) * 128],
                            start=(kt == 0), stop=(kt == KT - 1))
                    # both heads in one strided copy:
                    # [128, 2, 64] -> v_sb cols [blk+1:blk+65],[blk+66:blk+130]
                    nc.vector.tensor_copy(
                        v_sb[bb2][:, ttb * VROW:(ttb + 1) * VROW]
                        .rearrange("p (h c) -> p h c", h=2)[:, :, 1:HD + 1],
                        vp.rearrange("p (h c) -> p h c", h=2))

                # ---- projection micro-pieces -------------------------------
                # Each piece is a small closure; cost is approximate PE-µs.
                def fill_pieces(bb, with_loads=True):
                    pieces = []
                    for ntb in range(NTB):
                        nt = bb * NTB + ntb
                        if with_loads:
                            pieces.append((0.05, lambda nt=nt: load(nt),
                                           bb, ntb))

                        def piece(cost, fn, *a):
                            return (cost, lambda fn=fn, a=a: fn(*a), bb, ntb)

                        # the last batch's q blocks for ntb>=1 are first read
                        # by wave (B-1, ntb): deferred there to fill the
                        # otherwise-idle chain-bound waves
                        defer_q = (bb == B - 1 and ntb >= 1)
                        if not defer_q:
                            pieces += [
                                piece(0.9, qk_mm, nt, 0, 0),
                                piece(0.9, qk_mm, nt, 0, 1),
                                piece(0.0, qk_copy, nt, 0),
                            ]
                        # v pieces spaced between the two qk groups so the
                        # qkps buffer's DVE copy has drained before reuse
                        pieces += [
                            piece(0.5, v_mm, nt, 0),
                            piece(0.5, v_mm, nt, 1),
                            piece(0.9, qk_mm, nt, 1, 0),
                            piece(0.9, qk_mm, nt, 1, 1),
                            piece(0.0, qk_copy, nt, 1),
                            piece(0.5, v_mm, nt, 2),
                            piece(0.5, v_mm, nt, 3),
                        ]
                    return pieces

                def emit_tail(pb, pqt, av):
                    # batch per engine so DVE never head-of-line blocks on
                    # a gpsimd broadcast
                    rcs, bcss, ots = [], [], []
                    for h in range(2):
                        rc = rcp.tile([1, 512], F32, name=f"rc{h}", tag=f"rc{h}")
                        nc.vector.reciprocal_approx_fast(rc[0:1, :], av[h][0:1, :])
                        rcs.append(rc)
                    for h in range(2):
                        bcs = rcp.tile([65, 512], F32, name=f"bcs{h}", tag=f"bcs{h}")
                        nc.gpsimd.partition_broadcast(bcs[:, :], rcs[h][0:1, :])
                        bcss.append(bcs)
                    for h in range(2):
                        ot = outp.tile([65, 512], F32)
                        nc.vector.tensor_mul(ot[0:65, :], av[h][0:65, :],
                                             bcss[h][0:65, :])
                        ots.append(ot)
                    for h in range(2):
                        ot2 = outp.tile([65, 512], F32, name="ot2", tag="ot2")
                        nc.scalar.activation(ot2[0:65, :], ots[h][0:65, :],
                                             AF.Identity, bias=bvq_s[:, h:h + 1])
                        nc.sync.dma_start(
                            out=out[h, pb, pqt, :, :],
                            in_=ot2[1:65, :])

                from collections import deque
                filler = deque()
                credit = [0.0]

                def pop_fill(add):
                    credit[0] += add
                    while filler and credit[0] > 0:
                        cost, fn, _, _ = filler.popleft()
                        credit[0] -= cost
                        fn()
                    if not filler:
                        credit[0] = 0.0

                def ensure_proj(bb, upto_ntb):
                    # scores of wave (bb, 0) chunk kt read k_sb[bb] columns
                    # written by block kt//4 — emission order IS the data
                    # order for the tile scheduler, so force-drain those
                    # pieces before emitting the consumer
                    while filler and any(
                            p[2] == bb and p[3] <= upto_ntb for p in filler):
                        cost, fn, _, _ = filler.popleft()
                        credit[0] -= cost
                        fn()

                # batch 0: nt0's q/k groups hand-scheduled for the startup
                # critical path — k accumulates in the vps bank so the
                # k-group needs no wait on the q-copy's qkps buffer
                qps0 = qkps.tile([128, 512], F32, name="ps", tag="ps")
                kps0 = vps.tile([128, 512], F32, name="vp", tag="vp")
                xnt00 = xnt_tiles[0]
                for kh in range(2):
                    for mt in range(2):
                        ps0 = qps0 if mt == 0 else kps0
                        wmt0 = wq_s if mt == 0 else wk_s
                        for kt in range(kh * 4, kh * 4 + 4):
                            nc.tensor.matmul(
                                ps0[:, :],
                                lhsT=wmt0[:, kt * 128:(kt + 1) * 128],
                                rhs=xnt00[:, kt * 512:(kt + 1) * 512],
                                start=(kt == 0), stop=(kt == KT - 1))
                for mt in range(2):
                    nc.vector.tensor_scalar_add(
                        (q_sb[0] if mt == 0 else k_sb[0])[:, 0:512],
                        (qps0 if mt == 0 else kps0)[:, :],
                        bqk_s[:, mt:mt + 1])
                b0_pieces = fill_pieces(0, with_loads=False)
                for cost, fn, _, _ in b0_pieces[3:5] + b0_pieces[8:10]:
                    fn()                                 # nt0 v pieces
                filler.extend(b0_pieces[10:])

                for b in range(B):
                    for qt in range(QT):
                        if qt == 1 and b + 1 < B:
                            filler.extend(fill_pieces(b + 1))
                        qcol = qt * 512
                        if b == B - 1 and qt >= 1:
                            # deferred q block for this wave (see fill_pieces)
                            nt_d = b * NTB + qt
                            qk_mm(nt_d, 0, 0)
                            qk_mm(nt_d, 0, 1)
                            qk_copy(nt_d, 0)
                        pav = [avps.tile([65, 512], F32, name=f"av{_h}",
                                         tag=f"av{_h}", bufs=1)
                               for _h in range(2)]

                        def av_mm(kt):
                            for h in range(2):
                                nc.tensor.matmul(
                                    pav[h][:, :],
                                    lhsT=v_sb[b][:, kt * VROW + h * (HD + 1):
                                                 kt * VROW + (h + 1) * (HD + 1)],
                                    rhs=echunks[kt][:, h * 512:(h + 1) * 512],
                                    start=(kt == 0), stop=(kt == KTOK_B - 1),
                                    skip_group_check=True)

                        echunks = []
                        for kt2 in range(0, KTOK_B, 2):
                            # two chunks of scores back-to-back: one stationary
                            # switch-in per pair instead of per chunk
                            for kt in (kt2, kt2 + 1):
                                kcol = kt * 128
                                if qt == 0:
                                    ensure_proj(b, kt // 4)
                                s2 = sps.tile([128, 1024], F32, name="s2",
                                              tag="s2")
                                for h in range(2):
                                    nc.tensor.matmul(
                                        s2[:, h * 512:(h + 1) * 512],
                                        lhsT=k_sb[b][h * 64:(h + 1) * 64,
                                                     kcol:kcol + 128],
                                        rhs=q_sb[b][h * 64:(h + 1) * 64,
                                                    qcol:qcol + 512],
                                        start=True, stop=True,
                                        tile_position=(h * 64, 0))
                                e2 = expp.tile([128, 1024], BF16, name="e2",
                                               tag="e2")
                                # last wave: no filler left, its exp chain is
                                # the endgame — rebalance toward DVE so both
                                # exp engines finish together (~6.2us each)
                                last_wave = (b, qt) == (B - 1, QT - 1)
                                use_dve = (kt % 3 != 0) if last_wave \
                                    else (kt % 2 == 1)
                                if use_dve:
                                    nc.vector.tensor_scalar(
                                        out=e2[:, :].bitcast(I16), in0=s2[:, :],
                                        scalar1=SCH_A, scalar2=SCH_B,
                                        op0=ALU.mult, op1=ALU.add)
                                else:
                                    nc.scalar.activation(e2[:, :], s2[:, :],
                                                         AF.Exp)
                                echunks.append(e2)
                            # four AV matmuls trail as one block
                            if kt2 >= AV_LAG:
                                av_mm(kt2 - AV_LAG)
                                av_mm(kt2 - AV_LAG + 1)
                            gap_budget = 3.0 if (b, qt) == (0, 0) else 0.7
                            pop_fill(gap_budget if kt2 < KTOK_B - 2 else 0.0)
                        budget = 0.6 if (b, qt) != (B - 1, QT - 1) else 1e9
                        pop_fill(budget)
                        for kt in range(KTOK_B - AV_LAG, KTOK_B):
                            av_mm(kt)
                        emit_tail(b, qt, pav)
    nc.compile()
    return nc


_GRAPH = None


def _get_graph():
    global _GRAPH
    if _GRAPH is None:
        _GRAPH = build_graph()
    return _GRAPH


def _part_major(w_t):
    # [DIM, F] (x-dim major) -> [128, KT*F]: per-partition kt-blocks
    # contiguous so each partition is one DMA descriptor
    f = w_t.shape[1]
    return np.ascontiguousarray(
        w_t.reshape(KT, 128, f).transpose(1, 0, 2).reshape(128, KT * f))


def _make_in_maps(x, w_qkv, b_qkv):
    bf = ml_dtypes.bfloat16
    xt = x.reshape(TOK, DIM).T          # [DIM, TOK]
    # [128, nt, kt, 512]: per (partition, tile) an 8KB contiguous block
    xtr = np.ascontiguousarray(
        xt.reshape(KT, 128, NT, 512).transpose(1, 2, 0, 3)
        .reshape(128, NT * KT * 512)).astype(bf)
    in_maps = []
    for c in range(NCORES):
        hA, hB = HPC * c, HPC * c + 1
        rq = [w_qkv[h * HD:(h + 1) * HD] * SCALE for h in (hA, hB)]
        rk = [w_qkv[DIM + h * HD: DIM + (h + 1) * HD] for h in (hA, hB)]
        rv = [w_qkv[2 * DIM + h * HD: 2 * DIM + (h + 1) * HD] for h in (hA, hB)]
        wq_c = _part_major(np.concatenate(rq, axis=0).T).astype(bf)
        wk_c = _part_major(np.concatenate(rk, axis=0).T).astype(bf)
        wv_c = _part_major(np.concatenate(rv, axis=0).T).astype(bf)
        bq = [b_qkv[h * HD:(h + 1) * HD] * SCALE for h in (hA, hB)]
        bk = [b_qkv[DIM + h * HD: DIM + (h + 1) * HD] for h in (hA, hB)]
        bvc = [b_qkv[2 * DIM + h * HD: 2 * DIM + (h + 1) * HD] for h in (hA, hB)]
        bqk_c = np.concatenate(bq + bk).astype(np.float32).reshape(-1, 1)
        bvq_c = np.zeros((HD + 1, HPC), dtype=np.float32)
        for hh in range(HPC):
            bvq_c[1:HD + 1, hh] = bvc[hh]
        in_maps.append({"xt": xtr, "wq": wq_c, "wk": wk_c, "wv": wv_c,
                        "bqk": np.ascontiguousarray(bqk_c),
                        "bvq": bvq_c})
    return in_maps


def _run(x, w_qkv, b_qkv, trace=False, tmpdir=None):
    nc = _get_graph()
    in_maps = _make_in_maps(np.asarray(x, dtype=np.float32),
                            np.asarray(w_qkv, dtype=np.float32),
                            np.asarray(b_qkv, dtype=np.float32))
    res = run_bass_kernel_spmd(nc, in_maps, core_ids=list(range(NCORES)),
                               trace=trace, tmpdir=tmpdir)
    full = np.empty((B, N, DIM), dtype=np.float32)
    for c in range(NCORES):
        oc = res.results[c]["out"]          # [HPC, B, QT, HD, 512]
        # out[b, qt*512+j, (HPC*c+hh)*HD + d] = oc[hh, b, qt, d, j]
        full[:, :, c * HPC * HD:(c + 1) * HPC * HD] = \
            oc.transpose(1, 2, 4, 0, 3).reshape(B, N, HPC * HD)
    return full, res


def kernel(x, w_qkv, b_qkv):
    full, _ = _run(x, w_qkv, b_qkv, trace=False)
    return full



# revision 30
# speedup vs baseline: 1.0210x; 1.0048x over previous
"""Multi-head attention (B=4, N=2048, DIM=1024, H=16, HD=64) on 8 TRN2 cores.

Sharding: tensor-parallel over heads — 2 heads per core. The reference omits
the output projection, so each core's output is a disjoint 128-column slice of
the final [B, N, 1024]; no collectives are needed.

Per-core device kernel (bf16 compute, fp32 PSUM accumulation):
  - QKV projection from a single pass over x^T: q^T,k^T produced transposed
    [outch, tokens] (weights stationary), v produced natural [tokens, outch]
    (x tiles stationary).
  - scores^T = k^T.T @ q^T per (batch, head): K=64 contraction; head A lives
    on partitions 0-63 and head B on 64-127, so the two heads' matmuls
    row-tile the PE array and run concurrently.
  - exp split across two engines: most kt-chunks on ScalarE (table exp),
    the rest on DVE via a Schraudolph-style bit-trick (i16 = a*s + b,
    bitcast to bf16), which lands within the error tolerance and removes
    the ScalarE bottleneck.
  - out^T = [1 | v]^T @ expT accumulated over k tiles; row 0 is the softmax
    denominator. Tail: DVE fast reciprocal straight from PSUM row 0, GpSimd
    partition-broadcast, DVE multiply, GpSimd per-partition bias add, DMA out.
  - Projection work is emitted as micro-pieces interleaved into the score
    stream so the PE fills the exp-paced gaps instead of idling.
"""

import numpy as np
import ml_dtypes

import concourse.bacc as bacc
import concourse.mybir as mybir
from concourse.bass_utils import run_bass_kernel_spmd
from concourse.tile import TileContext

B, N, DIM, H = 4, 2048, 1024, 16
HD = DIM // H
SCALE = 1.0 / np.sqrt(HD)
TOK = B * N               # 8192 tokens
NCORES = 8
HPC = H // NCORES         # heads per core = 2

BF16 = mybir.dt.bfloat16
F32 = mybir.dt.float32
I16 = mybir.dt.int16
AF = mybir.ActivationFunctionType
ALU = mybir.AluOpType


NT = TOK // 512           # 16 token tiles of 512 for the projection
KT = 8                    # 1024 / 128 contraction tiles
QT = N // 512             # 4 q tiles per (b, h)
KTOK = N // 128           # 16 k-token tiles per (b, h)
VROW = 2 * (HD + 1)       # 130: [1 | vA | 1 | vB] per token tile

# Schraudolph bf16 exp: i16 = trunc(A*s + B), bitcast to bf16.
# A = 128/ln2 (SCALE folded into wq on host). B tuned for truncation.
SCH_A = 184.6650309
SCH_B = 16248.5

# kt chunks computed on DVE (bit-trick); rest on ScalarE table exp. Measured:
# DVE chunk [128,1024] = ~1210ns (psum fp32 read is the bottleneck, no 2x
# 16-bit path), ScalarE = ~1042ns; DVE also carries copies/casts/tail, so the
# balance point is ~6 DVE / 10 ScalarE
DVE_CHUNKS = frozenset((2, 4, 7, 9, 12, 14))
# AV matmuls trail the scores stream by AV_LAG chunks (even: chunk-pair
# granularity) so the PE switches stationary-weight streams half as often
AV_LAG = 6


def build_graph():
    nc = bacc.Bacc("TRN2", target_bir_lowering=False, debug=False)
    # host pre-arranges x/weights partition-major so every per-partition
    # tile block is one contiguous DMA descriptor (8KB for x tiles) instead
    # of 8x 1KB strided rows — the DMA engines are descriptor-rate bound
    xt = nc.declare_dram_parameter("xt", [128, NT * KT * 512], BF16,
                                   isOutput=False)
    wq = nc.declare_dram_parameter("wq", [128, KT * 128], BF16, isOutput=False)
    wk = nc.declare_dram_parameter("wk", [128, KT * 128], BF16, isOutput=False)
    wv = nc.declare_dram_parameter("wv", [128, KT * 128], BF16, isOutput=False)
    bqk = nc.declare_dram_parameter("bqk", [2 * HPC * HD, 1], F32, isOutput=False)
    bvq = nc.declare_dram_parameter("bvq", [HD + 1, HPC], F32, isOutput=False)
    out = nc.declare_dram_parameter("out", [HPC, B, N // 512, HD, 512], F32,
                                    isOutput=True)
    NTB = N // 512            # 4 proj token-tiles per batch
    KTOK_B = N // 128         # 16 k-token tiles per batch

    with TileContext(nc) as tc:
        with (
            tc.tile_pool(name="const", bufs=1) as constp,
            tc.tile_pool(name="qk", bufs=1) as qkp,
            tc.tile_pool(name="xin", bufs=4) as xinp,
            tc.tile_pool(name="exps", bufs=32) as expp,
            tc.tile_pool(name="outs", bufs=6) as outp,
            tc.tile_pool(name="rcs", bufs=2) as rcp,
        ):
            # ---- first x tile first: it gates the first matmul, and the
            # serial Sync trigger queue issues DMAs in program order
            xnt_tiles = {}

            def load(nt):
                xnt = xinp.tile([128, KT * 512], BF16, name="xnt")
                nc.sync.dma_start(out=xnt[:, :],
                                  in_=xt[:, nt * 4096:(nt + 1) * 4096])
                xnt_tiles[nt] = xnt

            # startup-critical bytes in dependency order on the 16-engine
            # Sync queue (other engines' queues measured ~4x slower): the
            # first q-group needs xnt0 kt0-3 + wq; the k-group adds wk and
            # the second x half; wv/v_mm come later
            xnt0 = xinp.tile([128, KT * 512], BF16, name="xnt")
            wq_s = constp.tile([128, KT * 128], BF16)
            wk_s = constp.tile([128, KT * 128], BF16)
            wv_s = constp.tile([128, KT * 128], BF16)
            # warm tile first on the GpSimd queue: it gates the PE clock
            # warm-up and must land before the bias DMA triggers
            warm = constp.tile([128, 128], BF16)
            nc.gpsimd.memset(warm[:, :], 0.25)
            nc.sync.dma_start(out=xnt0[:, 0:2048], in_=xt[:, 0:2048])
            nc.sync.dma_start(out=wq_s[:, :], in_=wq[:, :])
            nc.sync.dma_start(out=wk_s[:, :], in_=wk[:, :])
            nc.sync.dma_start(out=xnt0[:, 2048:4096], in_=xt[:, 2048:4096])
            nc.sync.dma_start(out=wv_s[:, :], in_=wv[:, :])
            xnt_tiles[0] = xnt0
            # bias transfers are many tiny descriptors: keep them off the
            # Sync queue so they don't delay the x tiles
            bqk_s = constp.tile([128, 2], F32)
            nc.gpsimd.dma_start(out=bqk_s[:, 0:1], in_=bqk[0:128, :])
            nc.gpsimd.dma_start(out=bqk_s[:, 1:2], in_=bqk[128:256, :])
            bvq_s = constp.tile([HD + 1, HPC], F32)
            nc.gpsimd.dma_start(out=bvq_s[:, :], in_=bvq[:, :])
            for nt0 in range(1, NTB):
                load(nt0)

            # per-batch activation tensors (lets attention on batch b start
            # as soon as batch b's projection tiles land)
            q_sb = [qkp.tile([128, N], BF16, name=f"q_sb{_b}") for _b in range(B)]
            k_sb = [qkp.tile([128, N], BF16, name=f"k_sb{_b}") for _b in range(B)]
            v_sb = [qkp.tile([128, KTOK_B * VROW], BF16, name=f"v_sb{_b}") for _b in range(B)]
            # memsets on GpSimd (idle at startup) so DVE is free immediately
            for _b in range(B):
                nc.gpsimd.memset(v_sb[_b][:, :], 1.0)

            with (
                tc.tile_pool(name="qkps", bufs=1, space="PSUM") as qkps,
                tc.tile_pool(name="vps", bufs=1, space="PSUM") as vps,
                tc.tile_pool(name="sps", bufs=2, space="PSUM") as sps,
                tc.tile_pool(name="avps", bufs=1, space="PSUM") as avps,
            ):
                # PE p-state warm-up: ~6µs of junk matmuls keep the array
                # busy while the first x/w DMAs land, so real matmuls start
                # at full clock instead of the 0.65 GHz cold state
                wps = vps.tile([128, 128], F32, name="vp", tag="vp")
                NWARM = 52
                for wi in range(NWARM):
                    nc.tensor.matmul(wps[:, :], lhsT=warm[:, :], rhs=warm[:, :],
                                     start=(wi == 0), stop=(wi == NWARM - 1))

                qk_ps_live = {}

                def qk_mm(nt, mt, kh):
                    xnt = xnt_tiles[nt]
                    if kh == 0:
                        ps = qkps.tile([128, 512], F32, name="ps", tag="ps")
                        qk_ps_live[(nt, mt)] = ps
                    else:
                        ps = qk_ps_live[(nt, mt)]
                    wmt = wq_s if mt == 0 else wk_s
                    for kt in range(kh * 4, kh * 4 + 4):
                        nc.tensor.matmul(
                            ps[:, :],
                            lhsT=wmt[:, kt * 128:(kt + 1) * 128],
                            rhs=xnt[:, kt * 512:(kt + 1) * 512],
                            start=(kt == 0), stop=(kt == KT - 1))

                def qk_copy(nt, mt):
                    bb2, ntb2 = nt // NTB, nt % NTB
                    ps = qk_ps_live.pop((nt, mt))
                    dst = q_sb[bb2] if mt == 0 else k_sb[bb2]
                    nc.vector.tensor_scalar_add(
                        dst[:, ntb2 * 512:(ntb2 + 1) * 512], ps[:, :],
                        bqk_s[:, mt:mt + 1])

                def v_mm(nt, sub):
                    bb2, ntb2 = nt // NTB, nt % NTB
                    xnt = xnt_tiles[nt]
                    ttb = ntb2 * 4 + sub
                    vp = vps.tile([128, 128], F32, name="vp", tag="vp")
                    for kt in range(KT):
                        nc.tensor.matmul(
                            vp[:, :],
                            lhsT=xnt[:, kt * 512 + sub * 128:
                                     kt * 512 + (sub + 1) * 128],
                            rhs=wv_s[:, kt * 128:(kt + 1) * 128],
                            start=(kt == 0), stop=(kt == KT - 1))
                    # both heads in one strided copy:
                    # [128, 2, 64] -> v_sb cols [blk+1:blk+65],[blk+66:blk+130]
                    nc.vector.tensor_copy(
                        v_sb[bb2][:, ttb * VROW:(ttb + 1) * VROW]
                        .rearrange("p (h c) -> p h c", h=2)[:, :, 1:HD + 1],
                        vp.rearrange("p (h c) -> p h c", h=2))

                # ---- projection micro-pieces -------------------------------
                # Each piece is a small closure; cost is approximate PE-µs.
                def fill_pieces(bb, with_loads=True):
                    pieces = []
                    for ntb in range(NTB):
                        nt = bb * NTB + ntb
                        if with_loads:
                            pieces.append((0.05, lambda nt=nt: load(nt),
                                           bb, ntb))

                        def piece(cost, fn, *a):
                            return (cost, lambda fn=fn, a=a: fn(*a), bb, ntb)

                        # the last batch's q blocks for ntb>=1 are first read
                        # by wave (B-1, ntb): deferred there to fill the
                        # otherwise-idle chain-bound waves
                        defer_q = (bb == B - 1 and ntb >= 1)
                        if not defer_q:
                            pieces += [
                                piece(0.9, qk_mm, nt, 0, 0),
                                piece(0.9, qk_mm, nt, 0, 1),
                                piece(0.0, qk_copy, nt, 0),
                            ]
                        # v pieces spaced between the two qk groups so the
                        # qkps buffer's DVE copy has drained before reuse
                        pieces += [
                            piece(0.5, v_mm, nt, 0),
                            piece(0.5, v_mm, nt, 1),
                            piece(0.9, qk_mm, nt, 1, 0),
                            piece(0.9, qk_mm, nt, 1, 1),
                            piece(0.0, qk_copy, nt, 1),
                            piece(0.5, v_mm, nt, 2),
                            piece(0.5, v_mm, nt, 3),
                        ]
                    return pieces

                def tail_pieces(pb, pqt, av):
                    # normalize chain split into 4 piece-groups so the DVE
                    # ops dribble between the next wave's exp chunks instead
                    # of head-of-line blocking them as one burst
                    rcs, bcss, ots = [], [], []

                    def p_rc():
                        for h in range(2):
                            rc = rcp.tile([1, 512], F32, name=f"rc{h}",
                                          tag=f"rc{h}")
                            nc.vector.reciprocal_approx_fast(rc[0:1, :],
                                                             av[h][0:1, :])
                            rcs.append(rc)

                    def p_bc():
                        for h in range(2):
                            bcs = rcp.tile([65, 512], F32, name=f"bcs{h}",
                                           tag=f"bcs{h}")
                            nc.gpsimd.partition_broadcast(bcs[:, :],
                                                          rcs[h][0:1, :])
                            bcss.append(bcs)

                    def p_mul():
                        for h in range(2):
                            ot = outp.tile([65, 512], F32)
                            nc.vector.tensor_mul(ot[0:65, :], av[h][0:65, :],
                                                 bcss[h][0:65, :])
                            ots.append(ot)

                    def p_out():
                        for h in range(2):
                            ot2 = outp.tile([65, 512], F32, name="ot2",
                                            tag="ot2")
                            nc.scalar.activation(ot2[0:65, :], ots[h][0:65, :],
                                                 AF.Identity,
                                                 bias=bvq_s[:, h:h + 1])
                            nc.sync.dma_start(out=out[h, pb, pqt, :, :],
                                              in_=ot2[1:65, :])

                    return [p_rc, p_bc, p_mul, p_out]

                from collections import deque
                filler = deque()
                credit = [0.0]

                def pop_fill(add):
                    credit[0] += add
                    while filler and credit[0] > 0:
                        cost, fn, _, _ = filler.popleft()
                        credit[0] -= cost
                        fn()
                    if not filler:
                        credit[0] = 0.0

                def ensure_proj(bb, upto_ntb):
                    # scores of wave (bb, 0) chunk kt read k_sb[bb] columns
                    # written by block kt//4 — emission order IS the data
                    # order for the tile scheduler, so force-drain those
                    # pieces before emitting the consumer
                    while filler and any(
                            p[2] == bb and p[3] <= upto_ntb for p in filler):
                        cost, fn, _, _ = filler.popleft()
                        credit[0] -= cost
                        fn()

                # batch 0: nt0's q/k groups hand-scheduled for the startup
                # critical path — k accumulates in the vps bank so the
                # k-group needs no wait on the q-copy's qkps buffer
                qps0 = qkps.tile([128, 512], F32, name="ps", tag="ps")
                kps0 = vps.tile([128, 512], F32, name="vp", tag="vp")
                xnt00 = xnt_tiles[0]
                for kh in range(2):
                    for mt in range(2):
                        ps0 = qps0 if mt == 0 else kps0
                        wmt0 = wq_s if mt == 0 else wk_s
                        for kt in range(kh * 4, kh * 4 + 4):
                            nc.tensor.matmul(
                                ps0[:, :],
                                lhsT=wmt0[:, kt * 128:(kt + 1) * 128],
                                rhs=xnt00[:, kt * 512:(kt + 1) * 512],
                                start=(kt == 0), stop=(kt == KT - 1))
                for mt in range(2):
                    nc.vector.tensor_scalar_add(
                        (q_sb[0] if mt == 0 else k_sb[0])[:, 0:512],
                        (qps0 if mt == 0 else kps0)[:, :],
                        bqk_s[:, mt:mt + 1])
                b0_pieces = fill_pieces(0, with_loads=False)
                for cost, fn, _, _ in b0_pieces[3:5] + b0_pieces[8:10]:
                    fn()                                 # nt0 v pieces
                filler.extend(b0_pieces[10:])

                pending_tail = []
                for b in range(B):
                    for qt in range(QT):
                        if qt == 1 and b + 1 < B:
                            filler.extend(fill_pieces(b + 1))
                        qcol = qt * 512
                        if b == B - 1 and qt >= 1:
                            # deferred q block for this wave (see fill_pieces)
                            nt_d = b * NTB + qt
                            qk_mm(nt_d, 0, 0)
                            qk_mm(nt_d, 0, 1)
                            qk_copy(nt_d, 0)
                        pav = [avps.tile([65, 512], F32, name=f"av{_h}",
                                         tag=f"av{_h}", bufs=1)
                               for _h in range(2)]

                        def av_mm(kt):
                            for h in range(2):
                                nc.tensor.matmul(
                                    pav[h][:, :],
                                    lhsT=v_sb[b][:, kt * VROW + h * (HD + 1):
                                                 kt * VROW + (h + 1) * (HD + 1)],
                                    rhs=echunks[kt][:, h * 512:(h + 1) * 512],
                                    start=(kt == 0), stop=(kt == KTOK_B - 1),
                                    skip_group_check=True)

                        echunks = []
                        for kt2 in range(0, KTOK_B, 2):
                            # two chunks of scores back-to-back: one stationary
                            # switch-in per pair instead of per chunk
                            for kt in (kt2, kt2 + 1):
                                kcol = kt * 128
                                if qt == 0:
                                    ensure_proj(b, kt // 4)
                                s2 = sps.tile([128, 1024], F32, name="s2",
                                              tag="s2")
                                for h in range(2):
                                    nc.tensor.matmul(
                                        s2[:, h * 512:(h + 1) * 512],
                                        lhsT=k_sb[b][h * 64:(h + 1) * 64,
                                                     kcol:kcol + 128],
                                        rhs=q_sb[b][h * 64:(h + 1) * 64,
                                                    qcol:qcol + 512],
                                        start=True, stop=True,
                                        tile_position=(h * 64, 0))
                                e2 = expp.tile([128, 1024], BF16, name="e2",
                                               tag="e2")
                                if kt in DVE_CHUNKS:
                                    nc.vector.tensor_scalar(
                                        out=e2[:, :].bitcast(I16), in0=s2[:, :],
                                        scalar1=SCH_A, scalar2=SCH_B,
                                        op0=ALU.mult, op1=ALU.add)
                                else:
                                    nc.scalar.activation(e2[:, :], s2[:, :],
                                                         AF.Exp)
                                echunks.append(e2)
                            # previous wave's tail dribbles one piece-group
                            # per chunk-pair slot (all DVE/ScalarE tail ops
                            # land between exp chunks, never as a burst)
                            if pending_tail:
                                pending_tail.pop(0)()
                            # four AV matmuls trail as one block
                            if kt2 >= AV_LAG:
                                av_mm(kt2 - AV_LAG)
                                av_mm(kt2 - AV_LAG + 1)
                            gap_budget = 3.0 if (b, qt) == (0, 0) else 0.7
                            pop_fill(gap_budget if kt2 < KTOK_B - 2 else 0.0)
                        budget = 0.6 if (b, qt) != (B - 1, QT - 1) else 1e9
                        pop_fill(budget)
                        for kt in range(KTOK_B - AV_LAG, KTOK_B):
                            av_mm(kt)
                        pending_tail = tail_pieces(b, qt, pav)
                # final wave's tail runs immediately
                for piece in pending_tail:
                    piece()
    nc.compile()
    return nc


_GRAPH = None


def _get_graph():
    global _GRAPH
    if _GRAPH is None:
        _GRAPH = build_graph()
    return _GRAPH


def _part_major(w_t):
    # [DIM, F] (x-dim major) -> [128, KT*F]: per-partition kt-blocks
    # contiguous so each partition is one DMA descriptor
    f = w_t.shape[1]
    return np.ascontiguousarray(
        w_t.reshape(KT, 128, f).transpose(1, 0, 2).reshape(128, KT * f))


def _make_in_maps(x, w_qkv, b_qkv):
    bf = ml_dtypes.bfloat16
    xt = x.reshape(TOK, DIM).T          # [DIM, TOK]
    # [128, nt, kt, 512]: per (partition, tile) an 8KB contiguous block
    xtr = np.ascontiguousarray(
        xt.reshape(KT, 128, NT, 512).transpose(1, 2, 0, 3)
        .reshape(128, NT * KT * 512)).astype(bf)
    in_maps = []
    for c in range(NCORES):
        hA, hB = HPC * c, HPC * c + 1
        rq = [w_qkv[h * HD:(h + 1) * HD] * SCALE for h in (hA, hB)]
        rk = [w_qkv[DIM + h * HD: DIM + (h + 1) * HD] for h in (hA, hB)]
        rv = [w_qkv[2 * DIM + h * HD: 2 * DIM + (h + 1) * HD] for h in (hA, hB)]
        wq_c = _part_major(np.concatenate(rq, axis=0).T).astype(bf)
        wk_c = _part_major(np.concatenate(rk, axis=0).T).astype(bf)
        wv_c = _part_major(np.concatenate(rv, axis=0).T).astype(bf)
        bq = [b_qkv[h * HD:(h + 1) * HD] * SCALE for h in (hA, hB)]
        bk = [b_qkv[DIM + h * HD: DIM + (h + 1) * HD] for h in (hA, hB)]
        bvc = [b_qkv[2 * DIM + h * HD: 2 * DIM + (h + 1) * HD] for h in (hA, hB)]
        bqk_c = np.concatenate(bq + bk).astype(np.float32).reshape(-1, 1)
        bvq_c = np.zeros((HD + 1, HPC), dtype=np.float32)
        for hh in range(HPC):
            bvq_c[1:HD + 1, hh] = bvc[hh]
        in_maps.append({"xt": xtr, "wq": wq_c, "wk": wk_c, "wv": wv_c,
                        "bqk": np.ascontiguousarray(bqk_c),
                        "bvq": bvq_c})
    return in_maps


def _run(x, w_qkv, b_qkv, trace=False, tmpdir=None):
    nc = _get_graph()
    in_maps = _make_in_maps(np.asarray(x, dtype=np.float32),
                            np.asarray(w_qkv, dtype=np.float32),
                            np.asarray(b_qkv, dtype=np.float32))
    res = run_bass_kernel_spmd(nc, in_maps, core_ids=list(range(NCORES)),
                               trace=trace, tmpdir=tmpdir)
    full = np.empty((B, N, DIM), dtype=np.float32)
    for c in range(NCORES):
        oc = res.results[c]["out"]          # [HPC, B, QT, HD, 512]
        # out[b, qt*512+j, (HPC*c+hh)*HD + d] = oc[hh, b, qt, d, j]
        full[:, :, c * HPC * HD:(c + 1) * HPC * HD] = \
            oc.transpose(1, 2, 4, 0, 3).reshape(B, N, HPC * HD)
    return full, res


def kernel(x, w_qkv, b_qkv):
    full, _ = _run(x, w_qkv, b_qkv, trace=False)
    return full

